# revision 21
# baseline (speedup 1.0000x reference)
"""MoE (top-2 of 8 experts) Trainium2 kernel.

Strategy: expert-parallel across the 8 NeuronCores. The (cheap) router runs
on host CPU; the host gathers each expert's routed tokens, each core runs
the dense expert MLP  y = (silu(x @ w1_e) @ w2_e) * combine_weight  for its
expert's tokens only, and the host scatter-adds the per-expert
contributions back into the full [B,S,D] output.

Numerics: fp8(e4m3) DoubleRow matmuls with hi+lo residual operands.
Each operand v is split as v ~ q8(v) + q8(v - q8(v)); a matmul a@b is
computed as a_hi@b_hi + a_lo@b_hi + a_hi@b_lo (lo*lo dropped), giving
~8-bit effective mantissas at 1.5 fp8-DoubleRow chains per 256-deep
K-chunk -- 1.33x fewer PE cycles than bf16 at much better than the
required accuracy (measured ~2e-3 max rel err vs the 2e-2 gate).
Weights are pre-scaled by 64 so their magnitudes sit well inside e4m3's
normal range; the 1/64 is folded into the silu scale (mm1) and the
combine weights (mm2).

Self-contained: only environment packages (numpy/jax/ml_dtypes/concourse).
"""

import sys

import numpy as np

# concourse ships on sys.path via the container's sitecustomize
# (/root/.axon_site/_ro/trn_rl_repo); /opt copy is a fallback only.
if "/opt/trn_rl_repo" not in sys.path:
    sys.path.append("/opt/trn_rl_repo")

B, S, D_MODEL, D_FF, N_EXPERTS, TOP_K = 2, 2048, 1024, 2048, 8, 2
T = B * S
N_CORES = 8
WS = 64.0           # weight pre-scale (power of 2: exact in fp)
K2 = D_MODEL // 256  # 4  K-pair chunks for matmul 1
F2 = D_FF // 256     # 8  K-pair chunks for matmul 2
ND = D_MODEL // 256  # 4  output column chunks for matmul 2

_PROGRAM_CACHE: dict = {}
_WPACK_CACHE: dict = {}
LAST_BUILD = {}


def _round_up(v: int, m: int) -> int:
    return ((v + m - 1) // m) * m


def _blocks(C: int, bs_max: int = 256):
    out = []
    b0 = 0
    while b0 < C:
        bs = min(bs_max, C - b0)
        out.append((b0, bs))
        b0 += bs
    return out


def _build_program(C: int):
    """Per-core expert-MLP program for token capacity C (C need not be a
    multiple of 128; tile shapes are padded to CP = roundup(C, 128))."""
    import concourse.tile as tile
    from concourse import bacc, mybir

    f8 = mybir.dt.float8e4
    f16 = mybir.dt.float16
    f32 = mybir.dt.float32
    silu = mybir.ActivationFunctionType.Silu
    DR = mybir.MatmulPerfMode.DoubleRow
    CP = _round_up(C, 128)

    NBP = (CP + 255) // 256  # 256-wide token block grid (x is block-major)

    nc = bacc.Bacc("TRN2", target_bir_lowering=False, debug=False,
                   num_devices=N_CORES)
    xh_d = nc.dram_tensor("xh", [NBP, 128, K2, 2, 256], f8,
                          kind="ExternalInput").ap()
    xl_d = nc.dram_tensor("xl", [NBP, 128, K2, 2, 256], f8,
                          kind="ExternalInput").ap()
    w1h_d = nc.dram_tensor("w1h", [K2, 128, 2, D_FF], f8,
                           kind="ExternalInput").ap()
    w1l_d = nc.dram_tensor("w1l", [K2, 128, 2, D_FF], f8,
                           kind="ExternalInput").ap()
    w2h_d = nc.dram_tensor("w2h", [F2, 128, 2, D_MODEL], f8,
                           kind="ExternalInput").ap()
    w2l_d = nc.dram_tensor("w2l", [F2, 128, 2, D_MODEL], f8,
                           kind="ExternalInput").ap()
    # y is produced transposed, [d_model-chunk, 128, tokens]; the host
    # transposes back and applies the combine weights (and the 1/WS).
    NDM = D_MODEL // 128
    y_d = nc.dram_tensor("y", [NDM, 128, CP], f16, kind="ExternalOutput").ap()

    blocks = _blocks(C)

    with tile.TileContext(nc) as tc:
        with (
            tc.tile_pool(name="wpool", bufs=1) as wpool,
            tc.tile_pool(name="xpool", bufs=1) as xpool,
            tc.tile_pool(name="hpool", bufs=3) as hpool,
            tc.tile_pool(name="ypool", bufs=3) as ypool,
            tc.tile_pool(name="pspool", bufs=1, space="PSUM") as pspool,
        ):
            w1h_sb = [wpool.tile([128, 2, D_FF], f8, tag=f"w1h{k}",
                                 name=f"w1h{k}") for k in range(K2)]
            w1l_sb = [wpool.tile([128, 2, D_FF], f8, tag=f"w1l{k}",
                                 name=f"w1l{k}") for k in range(K2)]
            w2h_sb = [wpool.tile([128, 2, D_MODEL], f8, tag=f"w2h{k}",
                                 name=f"w2h{k}") for k in range(F2)]
            w2l_sb = [wpool.tile([128, 2, D_MODEL], f8, tag=f"w2l{k}",
                                 name=f"w2l{k}") for k in range(F2)]
            # per-block x tiles, [128, K2, 2, 256] (one contiguous DMA each)
            xh_sb, xl_sb = {}, {}

            def load_x(bi):
                xh_t = xpool.tile([128, K2, 2, 256], f8, tag="xbh", bufs=3,
                                  name="xbh")
                xl_t = xpool.tile([128, K2, 2, 256], f8, tag="xbl", bufs=3,
                                  name="xbl")
                nc.sync.dma_start(xh_t[:], xh_d[bi])
                nc.sync.dma_start(xl_t[:], xl_d[bi])
                xh_sb[bi] = xh_t
                xl_sb[bi] = xl_t

            # DMA order = first-use order given the chain schedule below.
            # w1 arrives in column halves so mm1 groups fm0-7 can close
            # (and evacuate) while fm8-15's weights are still in flight;
            # x arrives per 256-token block.
            # processing order: first two blocks (they pipeline against the
            # weight DMAs), then the small tail block (so its evac stalls
            # are absorbed mid-stream), then the rest.
            order = list(range(len(blocks)))
            if len(order) >= 4:
                order = order[:2] + [order[-1]] + order[2:-1]

            HF = D_FF // 2
            nc.sync.dma_start(w1h_sb[0][:, :, 0:HF], w1h_d[0][:, :, 0:HF])
            load_x(order[0])
            for k in range(1, K2):
                nc.sync.dma_start(w1h_sb[k][:, :, 0:HF], w1h_d[k][:, :, 0:HF])
            for k in range(K2):
                nc.sync.dma_start(w1l_sb[k][:, :, 0:HF], w1l_d[k][:, :, 0:HF])
            if len(order) > 1:
                load_x(order[1])
            for k in range(K2):
                nc.sync.dma_start(w1h_sb[k][:, :, HF:D_FF],
                                  w1h_d[k][:, :, HF:D_FF])
                nc.sync.dma_start(w1l_sb[k][:, :, HF:D_FF],
                                  w1l_d[k][:, :, HF:D_FF])
            for k in range(F2):
                nc.sync.dma_start(w2h_sb[k][:], w2h_d[k])
            for k in range(F2):
                nc.sync.dma_start(w2l_sb[k][:], w2l_d[k])
            for bi in order[2:]:
                load_x(bi)

            def emit_mm1(bi, b0, bs, fm_lo, fm_hi, hts):
                # --- mm1: h16 = silu((x @ w1) / WS), stored as pair tiles
                # [128, 2, bs] per 256-wide ff chunk; hh/hl are the fp8
                # hi/lo split feeding mm2's stationary operand. Emitted in
                # fm-half passes matching the w1 column-half DMA arrival.
                h16_sb, hh_sb, hl_sb = hts
                for k in range(fm_lo // 2, (fm_hi + 1) // 2):
                    h16_sb[k] = hpool.tile([128, 2, 256], f16, tag=f"h16_{k}",
                                           name=f"h16_{k}")
                    hh_sb[k] = hpool.tile([128, 2, 256], f8, tag=f"hh{k}",
                                          name=f"hh{k}")
                    hl_sb[k] = hpool.tile([128, 2, 256], f8, tag=f"hl{k}",
                                          name=f"hl{k}")
                for fm in range(fm_lo, fm_hi):
                    ps = pspool.tile([128, 256], f32, tag="psh", bufs=4,
                                     name="psh")
                    fmc = slice(fm * 128, fm * 128 + 128)
                    n = 3 * K2
                    i = 0
                    # chain-major order: A (w1h*xh), B (w1l*xh), C (w1h*xl)
                    # so late-landing DMAs (w1l, xl) gate only the later
                    # instructions of each accumulation group.
                    for lhs_set, rhs_t in (
                        (w1h_sb, xh_sb[bi]), (w1l_sb, xh_sb[bi]),
                        (w1h_sb, xl_sb[bi]),
                    ):
                        for k in range(K2):
                            nc.tensor.matmul(ps[:, :bs],
                                             lhsT=lhs_set[k][:, :, fmc],
                                             rhs=rhs_t[:, k, :, :bs],
                                             start=(i == 0), stop=(i == n - 1),
                                             perf_mode=DR)
                            i += 1
                    f2c, slot = fm // 2, fm % 2
                    nc.scalar.activation(h16_sb[f2c][:, slot, :bs], ps[:, :bs],
                                         silu, scale=1.0 / WS)
                    nc.vector.tensor_copy(hh_sb[f2c][:, slot, :bs],
                                          h16_sb[f2c][:, slot, :bs])
                    nc.vector.tensor_sub(hl_sb[f2c][:, slot, :bs],
                                         h16_sb[f2c][:, slot, :bs],
                                         hh_sb[f2c][:, slot, :bs])

            def emit_mm2(b0, bs, hh_sb, hl_sb):
                # --- mm2 (transposed): yT[d, t] = (h @ w2)[t, d].
                # Stationary = w2 128-column slices, moving = h token tiles,
                # so the tail block costs cycles proportional to its tokens.
                bc = slice(b0, b0 + bs)
                for dm in range(NDM):
                    dmc = slice(dm * 128, dm * 128 + 128)
                    py = pspool.tile([128, 256], f32, tag="psy", bufs=4,
                                     name="psy")
                    n = 3 * F2
                    i = 0
                    # chain-major: A (w2h*hh), B (w2h*hl), C (w2l*hh) --
                    # w2l is the last DMA to land.
                    for lhs_set, rhs_set in (
                        (w2h_sb, hh_sb), (w2h_sb, hl_sb), (w2l_sb, hh_sb),
                    ):
                        for k in range(F2):
                            nc.tensor.matmul(py[:, :bs],
                                             lhsT=lhs_set[k][:, :, dmc],
                                             rhs=rhs_set[k][:, :, :bs],
                                             start=(i == 0),
                                             stop=(i == n - 1),
                                             perf_mode=DR)
                            i += 1
                    ys = ypool.tile([128, 256], f16, tag="y", name="ys")
                    nc.scalar.copy(ys[:, :bs], py[:, :bs])
                    nc.sync.dma_start(y_d[dm][:, bc], ys[:, :bs])

            # Software pipeline. Prologue: fm-half passes over the first
            # two blocks interleaved with the w1 half DMAs; then steady
            # state mm2(b) followed by mm1(b+2) (depth 2).
            NF = D_FF // 128
            hs = {bi: ([None] * F2, [None] * F2, [None] * F2)
                  for bi in range(len(blocks))}
            pre = order[:2]
            for bi in pre:
                emit_mm1(bi, *blocks[bi], 0, NF // 2, hs[bi])
            for bi in pre:
                emit_mm1(bi, *blocks[bi], NF // 2, NF, hs[bi])
            for idx, bi in enumerate(order):
                if idx + 2 < len(order):
                    nb = order[idx + 2]
                    emit_mm1(nb, *blocks[nb], 0, NF, hs[nb])
                emit_mm2(*blocks[bi], hs[bi][1], hs[bi][2])

    nc.compile()
    return nc


def _route(x: np.ndarray, gate_w: np.ndarray):
    """Router on host CPU with the reference's exact jax ops/dtypes."""
    try:
        import jax
        import jax.numpy as jnp
        with jax.default_device(jax.devices("cpu")[0]):
            logits = jnp.einsum('bsd,de->bse', jnp.asarray(x),
                                jnp.asarray(gate_w))
            top_logits, top_idx = jax.lax.top_k(logits, TOP_K)
            top_w = jax.nn.softmax(top_logits, axis=-1)
            ti = np.asarray(top_idx).reshape(T, TOP_K)
            tw = np.asarray(top_w).reshape(T, TOP_K).astype(np.float32)
    except Exception:
        # numpy fallback (same selection semantics as jax.lax.top_k)
        logits = (x.reshape(T, D_MODEL) @ gate_w).astype(np.float32)
        i0 = np.argmax(logits, axis=1)
        masked = logits.copy()
        masked[np.arange(T), i0] = -np.inf
        i1 = np.argmax(masked, axis=1)
        v0 = logits[np.arange(T), i0]
        v1 = logits[np.arange(T), i1]
        e1 = np.exp(v1 - v0)
        w0 = 1.0 / (1.0 + e1)
        ti = np.stack([i0, i1], 1)
        tw = np.stack([w0, 1.0 - w0], 1).astype(np.float32)
    return ti, tw


def _pack_pairs(m: np.ndarray, free: int) -> np.ndarray:
    """[K, free] -> [K//256, 128, 2, free]: K-subtile pairs for DoubleRow."""
    K = m.shape[0]
    return np.ascontiguousarray(
        m.reshape(K // 256, 2, 128, free).transpose(0, 2, 1, 3))


def _pack_x_blocks(m: np.ndarray) -> np.ndarray:
    """[D, CP] -> [CP//256, 128, D//256, 2, 256]: token-block-major pair
    layout (one contiguous DMA per 256-token block)."""
    D, CP = m.shape
    return np.ascontiguousarray(
        m.reshape(D // 256, 2, 128, CP // 256, 256).transpose(3, 2, 0, 1, 4))


def _split8(a: np.ndarray, nf8):
    """hi/lo fp8 residual split of a float32 array."""
    hi = a.astype(nf8)
    lo = (a - hi.astype(np.float32)).astype(nf8)
    return hi, lo


def _pack_weights(w1: np.ndarray, w2: np.ndarray, nf8):
    key = (w1.ctypes.data, w2.ctypes.data, w1.shape, w2.shape)
    if key in _WPACK_CACHE:
        return _WPACK_CACHE[key]
    packs = []
    for e in range(N_EXPERTS):
        w1h, w1l = _split8(w1[e] * WS, nf8)
        w2h, w2l = _split8(w2[e] * WS, nf8)
        packs.append({
            "w1h": _pack_pairs(w1h, D_FF),
            "w1l": _pack_pairs(w1l, D_FF),
            "w2h": _pack_pairs(w2h, D_MODEL),
            "w2l": _pack_pairs(w2l, D_MODEL),
        })
    _WPACK_CACHE.clear()
    _WPACK_CACHE[key] = packs
    return packs


def kernel(x: np.ndarray, gate_w: np.ndarray, w1: np.ndarray,
           w2: np.ndarray) -> np.ndarray:
    from concourse.bass_utils import run_bass_kernel_spmd
    import ml_dtypes

    nf8 = ml_dtypes.float8_e4m3

    x = np.ascontiguousarray(np.asarray(x, dtype=np.float32))
    gate_w = np.ascontiguousarray(np.asarray(gate_w, dtype=np.float32))
    w1 = np.ascontiguousarray(np.asarray(w1, dtype=np.float32))
    w2 = np.ascontiguousarray(np.asarray(w2, dtype=np.float32))

    ti, tw = _route(x, gate_w)

    x2d = x.reshape(T, D_MODEL)
    tokens, weights = [], []
    for e in range(N_EXPERTS):
        rows, ks = np.nonzero(ti == e)
        tokens.append(rows)
        weights.append(tw[rows, ks])
    counts = [len(t) for t in tokens]
    C = max(max(counts), 128)
    CP = _round_up(C, 128)

    CPX = _round_up(CP, 256)  # x block grid is 256-wide

    if C not in _PROGRAM_CACHE:
        _PROGRAM_CACHE[C] = _build_program(C)
    nc = _PROGRAM_CACHE[C]

    wpacks = _pack_weights(w1, w2, nf8)

    in_maps = []
    for e in range(N_EXPERTS):
        n = counts[e]
        xt = np.zeros((D_MODEL, CPX), dtype=np.float32)
        if n:
            xt[:, :n] = x2d[tokens[e]].T
        xh, xl = _split8(xt, nf8)
        m = {
            "xh": _pack_x_blocks(xh),
            "xl": _pack_x_blocks(xl),
        }
        m.update(wpacks[e])
        in_maps.append(m)

    res = run_bass_kernel_spmd(nc, in_maps, core_ids=list(range(N_CORES)))

    out2d = np.zeros((T, D_MODEL), dtype=np.float32)
    for e in range(N_EXPERTS):
        n = counts[e]
        if n:
            # y comes back transposed [d_model, tokens] and unscaled; apply
            # combine weight and the deferred 1/WS here.
            yt = res.results[e]["y"].reshape(D_MODEL, -1)[:, :n]
            out2d[tokens[e]] += (yt.astype(np.float32)
                                 * (weights[e] / WS)[None, :]).T

    LAST_BUILD["nc"] = nc
    LAST_BUILD["C"] = C
    return out2d.reshape(B, S, D_MODEL)


# revision 22
# speedup vs baseline: 1.0138x; 1.0138x over previous
"""MoE (top-2 of 8 experts) Trainium2 kernel.

Strategy: expert-parallel across the 8 NeuronCores. The (cheap) router runs
on host CPU; the host gathers each expert's routed tokens, each core runs
the dense expert MLP  y = (silu(x @ w1_e) @ w2_e) * combine_weight  for its
expert's tokens only, and the host scatter-adds the per-expert
contributions back into the full [B,S,D] output.

Numerics: fp8(e4m3) DoubleRow matmuls with hi+lo residual operands.
Each operand v is split as v ~ q8(v) + q8(v - q8(v)); a matmul a@b is
computed as a_hi@b_hi + a_lo@b_hi + a_hi@b_lo (lo*lo dropped), giving
~8-bit effective mantissas at 1.5 fp8-DoubleRow chains per 256-deep
K-chunk -- 1.33x fewer PE cycles than bf16 at much better than the
required accuracy (measured ~2e-3 max rel err vs the 2e-2 gate).
Weights are pre-scaled by 64 so their magnitudes sit well inside e4m3's
normal range; the 1/64 is folded into the silu scale (mm1) and the
combine weights (mm2).

Self-contained: only environment packages (numpy/jax/ml_dtypes/concourse).
"""

import sys

import numpy as np

# concourse ships on sys.path via the container's sitecustomize
# (/root/.axon_site/_ro/trn_rl_repo); /opt copy is a fallback only.
if "/opt/trn_rl_repo" not in sys.path:
    sys.path.append("/opt/trn_rl_repo")

B, S, D_MODEL, D_FF, N_EXPERTS, TOP_K = 2, 2048, 1024, 2048, 8, 2
T = B * S
N_CORES = 8
WS = 64.0           # weight pre-scale (power of 2: exact in fp)
K2 = D_MODEL // 256  # 4  K-pair chunks for matmul 1
F2 = D_FF // 256     # 8  K-pair chunks for matmul 2
ND = D_MODEL // 256  # 4  output column chunks for matmul 2

_PROGRAM_CACHE: dict = {}
_WPACK_CACHE: dict = {}
LAST_BUILD = {}


def _round_up(v: int, m: int) -> int:
    return ((v + m - 1) // m) * m


def _blocks(C: int, bs_max: int = 256):
    out = []
    b0 = 0
    while b0 < C:
        bs = min(bs_max, C - b0)
        out.append((b0, bs))
        b0 += bs
    return out


def _build_program(C: int):
    """Per-core expert-MLP program for token capacity C (C need not be a
    multiple of 128; tile shapes are padded to CP = roundup(C, 128))."""
    import concourse.tile as tile
    from concourse import bacc, mybir

    f8 = mybir.dt.float8e4
    f16 = mybir.dt.float16
    f32 = mybir.dt.float32
    silu = mybir.ActivationFunctionType.Silu
    DR = mybir.MatmulPerfMode.DoubleRow
    CP = _round_up(C, 128)

    NBP = (CP + 255) // 256  # 256-wide token block grid (x is block-major)

    nc = bacc.Bacc("TRN2", target_bir_lowering=False, debug=False,
                   num_devices=N_CORES)
    xh_d = nc.dram_tensor("xh", [NBP, 128, K2, 2, 256], f8,
                          kind="ExternalInput").ap()
    xl_d = nc.dram_tensor("xl", [NBP, 128, K2, 2, 256], f8,
                          kind="ExternalInput").ap()
    w1h_d = nc.dram_tensor("w1h", [K2, 128, 2, D_FF], f8,
                           kind="ExternalInput").ap()
    w1l_d = nc.dram_tensor("w1l", [K2, 128, 2, D_FF], f8,
                           kind="ExternalInput").ap()
    w2h_d = nc.dram_tensor("w2h", [F2, 128, 2, D_MODEL], f8,
                           kind="ExternalInput").ap()
    w2l_d = nc.dram_tensor("w2l", [F2, 128, 2, D_MODEL], f8,
                           kind="ExternalInput").ap()
    # y is produced transposed, [d_model-chunk, 128, tokens]; the host
    # transposes back and applies the combine weights (and the 1/WS).
    NDM = D_MODEL // 128
    y_d = nc.dram_tensor("y", [NDM, 128, CP], f16, kind="ExternalOutput").ap()

    blocks = _blocks(C)

    with tile.TileContext(nc) as tc:
        with (
            tc.tile_pool(name="wpool", bufs=1) as wpool,
            tc.tile_pool(name="xpool", bufs=1) as xpool,
            tc.tile_pool(name="hpool", bufs=3) as hpool,
            tc.tile_pool(name="ypool", bufs=3) as ypool,
            tc.tile_pool(name="pspool", bufs=1, space="PSUM") as pspool,
        ):
            w1h_sb = [wpool.tile([128, 2, D_FF], f8, tag=f"w1h{k}",
                                 name=f"w1h{k}") for k in range(K2)]
            w1l_sb = [wpool.tile([128, 2, D_FF], f8, tag=f"w1l{k}",
                                 name=f"w1l{k}") for k in range(K2)]
            w2h_sb = [wpool.tile([128, 2, D_MODEL], f8, tag=f"w2h{k}",
                                 name=f"w2h{k}") for k in range(F2)]
            w2l_sb = [wpool.tile([128, 2, D_MODEL], f8, tag=f"w2l{k}",
                                 name=f"w2l{k}") for k in range(F2)]
            # per-block x tiles, [128, K2, 2, 256] (one contiguous DMA each)
            xh_sb, xl_sb = {}, {}

            def load_x(bi):
                xh_t = xpool.tile([128, K2, 2, 256], f8, tag="xbh", bufs=3,
                                  name="xbh")
                xl_t = xpool.tile([128, K2, 2, 256], f8, tag="xbl", bufs=3,
                                  name="xbl")
                nc.sync.dma_start(xh_t[:], xh_d[bi])
                nc.sync.dma_start(xl_t[:], xl_d[bi])
                xh_sb[bi] = xh_t
                xl_sb[bi] = xl_t

            # DMA order = first-use order given the chain schedule below.
            # w1 arrives in column halves so mm1 groups fm0-7 can close
            # (and evacuate) while fm8-15's weights are still in flight;
            # x arrives per 256-token block.
            # processing order: first two blocks (they pipeline against the
            # weight DMAs), then the small tail block (so its evac stalls
            # are absorbed mid-stream), then the rest.
            order = list(range(len(blocks)))
            if len(order) >= 4:
                order = order[:2] + [order[-1]] + order[2:-1]

            HF = D_FF // 2
            nc.sync.dma_start(w1h_sb[0][:, :, 0:HF], w1h_d[0][:, :, 0:HF])
            load_x(order[0])
            for k in range(1, K2):
                nc.sync.dma_start(w1h_sb[k][:, :, 0:HF], w1h_d[k][:, :, 0:HF])
            for k in range(K2):
                nc.sync.dma_start(w1l_sb[k][:, :, 0:HF], w1l_d[k][:, :, 0:HF])
            if len(order) > 1:
                load_x(order[1])
            for k in range(K2):
                nc.sync.dma_start(w1h_sb[k][:, :, HF:D_FF],
                                  w1h_d[k][:, :, HF:D_FF])
                nc.sync.dma_start(w1l_sb[k][:, :, HF:D_FF],
                                  w1l_d[k][:, :, HF:D_FF])
            for k in range(F2):
                nc.sync.dma_start(w2h_sb[k][:], w2h_d[k])
            for k in range(F2):
                nc.sync.dma_start(w2l_sb[k][:], w2l_d[k])
            for bi in order[2:]:
                load_x(bi)

            def emit_mm1(bi, b0, bs, fm_lo, fm_hi, hts):
                # --- mm1: h16 = silu((x @ w1) / WS), stored as pair tiles
                # [128, 2, bs] per 256-wide ff chunk; hh/hl are the fp8
                # hi/lo split feeding mm2's stationary operand. Emitted in
                # fm-half passes matching the w1 column-half DMA arrival.
                h16_sb, hh_sb, hl_sb = hts
                for k in range(fm_lo // 2, (fm_hi + 1) // 2):
                    h16_sb[k] = hpool.tile([128, 2, 256], f16, tag=f"h16_{k}",
                                           name=f"h16_{k}")
                    hh_sb[k] = hpool.tile([128, 2, 256], f8, tag=f"hh{k}",
                                          name=f"hh{k}")
                    hl_sb[k] = hpool.tile([128, 2, 256], f8, tag=f"hl{k}",
                                          name=f"hl{k}")
                for fm in range(fm_lo, fm_hi):
                    ps = pspool.tile([128, 256], f32, tag="psh", bufs=4,
                                     name="psh")
                    fmc = slice(fm * 128, fm * 128 + 128)
                    n = 3 * K2
                    i = 0
                    # chain-major order: A (w1h*xh), B (w1l*xh), C (w1h*xl)
                    # so late-landing DMAs (w1l, xl) gate only the later
                    # instructions of each accumulation group.
                    for lhs_set, rhs_t in (
                        (w1h_sb, xh_sb[bi]), (w1l_sb, xh_sb[bi]),
                        (w1h_sb, xl_sb[bi]),
                    ):
                        for k in range(K2):
                            nc.tensor.matmul(ps[:, :bs],
                                             lhsT=lhs_set[k][:, :, fmc],
                                             rhs=rhs_t[:, k, :, :bs],
                                             start=(i == 0), stop=(i == n - 1),
                                             perf_mode=DR)
                            i += 1
                    f2c, slot = fm // 2, fm % 2
                    nc.scalar.activation(h16_sb[f2c][:, slot, :bs], ps[:, :bs],
                                         silu, scale=1.0 / WS)
                    nc.vector.tensor_copy(hh_sb[f2c][:, slot, :bs],
                                          h16_sb[f2c][:, slot, :bs])
                    nc.vector.tensor_sub(hl_sb[f2c][:, slot, :bs],
                                         h16_sb[f2c][:, slot, :bs],
                                         hh_sb[f2c][:, slot, :bs])

            def emit_mm2(b0, bs, hh_sb, hl_sb):
                # --- mm2 (transposed): yT[d, t] = (h @ w2)[t, d].
                # Stationary = w2 128-column slices, moving = h token tiles,
                # so the tail block costs cycles proportional to its tokens.
                bc = slice(b0, b0 + bs)
                for dm in range(NDM):
                    dmc = slice(dm * 128, dm * 128 + 128)
                    py = pspool.tile([128, 256], f32, tag="psy", bufs=4,
                                     name="psy")
                    n = 3 * F2
                    i = 0
                    # chain-major: A (w2h*hh), B (w2h*hl), C (w2l*hh) --
                    # w2l is the last DMA to land.
                    for lhs_set, rhs_set in (
                        (w2h_sb, hh_sb), (w2h_sb, hl_sb), (w2l_sb, hh_sb),
                    ):
                        for k in range(F2):
                            nc.tensor.matmul(py[:, :bs],
                                             lhsT=lhs_set[k][:, :, dmc],
                                             rhs=rhs_set[k][:, :, :bs],
                                             start=(i == 0),
                                             stop=(i == n - 1),
                                             perf_mode=DR)
                            i += 1
                    ys = ypool.tile([128, 256], f16, tag="y", name="ys")
                    nc.scalar.copy(ys[:, :bs], py[:, :bs])
                    nc.sync.dma_start(y_d[dm][:, bc], ys[:, :bs])

            # Software pipeline. Prologue: fm-half passes over the first
            # two blocks interleaved with the w1 half DMAs; then steady
            # state mm2(b) followed by mm1(b+2) (depth 2).
            NF = D_FF // 128
            hs = {bi: ([None] * F2, [None] * F2, [None] * F2)
                  for bi in range(len(blocks))}
            pre = order[:2]
            for bi in pre:
                emit_mm1(bi, *blocks[bi], 0, NF // 2, hs[bi])
            for bi in pre:
                emit_mm1(bi, *blocks[bi], NF // 2, NF, hs[bi])
            for idx, bi in enumerate(order):
                emit_mm2(*blocks[bi], hs[bi][1], hs[bi][2])
                if idx + 2 < len(order):
                    nb = order[idx + 2]
                    emit_mm1(nb, *blocks[nb], 0, NF, hs[nb])

    nc.compile()
    return nc


def _route(x: np.ndarray, gate_w: np.ndarray):
    """Router on host CPU with the reference's exact jax ops/dtypes."""
    try:
        import jax
        import jax.numpy as jnp
        with jax.default_device(jax.devices("cpu")[0]):
            logits = jnp.einsum('bsd,de->bse', jnp.asarray(x),
                                jnp.asarray(gate_w))
            top_logits, top_idx = jax.lax.top_k(logits, TOP_K)
            top_w = jax.nn.softmax(top_logits, axis=-1)
            ti = np.asarray(top_idx).reshape(T, TOP_K)
            tw = np.asarray(top_w).reshape(T, TOP_K).astype(np.float32)
    except Exception:
        # numpy fallback (same selection semantics as jax.lax.top_k)
        logits = (x.reshape(T, D_MODEL) @ gate_w).astype(np.float32)
        i0 = np.argmax(logits, axis=1)
        masked = logits.copy()
        masked[np.arange(T), i0] = -np.inf
        i1 = np.argmax(masked, axis=1)
        v0 = logits[np.arange(T), i0]
        v1 = logits[np.arange(T), i1]
        e1 = np.exp(v1 - v0)
        w0 = 1.0 / (1.0 + e1)
        ti = np.stack([i0, i1], 1)
        tw = np.stack([w0, 1.0 - w0], 1).astype(np.float32)
    return ti, tw


def _pack_pairs(m: np.ndarray, free: int) -> np.ndarray:
    """[K, free] -> [K//256, 128, 2, free]: K-subtile pairs for DoubleRow."""
    K = m.shape[0]
    return np.ascontiguousarray(
        m.reshape(K // 256, 2, 128, free).transpose(0, 2, 1, 3))


def _pack_x_blocks(m: np.ndarray) -> np.ndarray:
    """[D, CP] -> [CP//256, 128, D//256, 2, 256]: token-block-major pair
    layout (one contiguous DMA per 256-token block)."""
    D, CP = m.shape
    return np.ascontiguousarray(
        m.reshape(D // 256, 2, 128, CP // 256, 256).transpose(3, 2, 0, 1, 4))


def _split8(a: np.ndarray, nf8):
    """hi/lo fp8 residual split of a float32 array."""
    hi = a.astype(nf8)
    lo = (a - hi.astype(np.float32)).astype(nf8)
    return hi, lo


def _pack_weights(w1: np.ndarray, w2: np.ndarray, nf8):
    key = (w1.ctypes.data, w2.ctypes.data, w1.shape, w2.shape)
    if key in _WPACK_CACHE:
        return _WPACK_CACHE[key]
    packs = []
    for e in range(N_EXPERTS):
        w1h, w1l = _split8(w1[e] * WS, nf8)
        w2h, w2l = _split8(w2[e] * WS, nf8)
        packs.append({
            "w1h": _pack_pairs(w1h, D_FF),
            "w1l": _pack_pairs(w1l, D_FF),
            "w2h": _pack_pairs(w2h, D_MODEL),
            "w2l": _pack_pairs(w2l, D_MODEL),
        })
    _WPACK_CACHE.clear()
    _WPACK_CACHE[key] = packs
    return packs


def kernel(x: np.ndarray, gate_w: np.ndarray, w1: np.ndarray,
           w2: np.ndarray) -> np.ndarray:
    from concourse.bass_utils import run_bass_kernel_spmd
    import ml_dtypes

    nf8 = ml_dtypes.float8_e4m3

    x = np.ascontiguousarray(np.asarray(x, dtype=np.float32))
    gate_w = np.ascontiguousarray(np.asarray(gate_w, dtype=np.float32))
    w1 = np.ascontiguousarray(np.asarray(w1, dtype=np.float32))
    w2 = np.ascontiguousarray(np.asarray(w2, dtype=np.float32))

    ti, tw = _route(x, gate_w)

    x2d = x.reshape(T, D_MODEL)
    tokens, weights = [], []
    for e in range(N_EXPERTS):
        rows, ks = np.nonzero(ti == e)
        tokens.append(rows)
        weights.append(tw[rows, ks])
    counts = [len(t) for t in tokens]
    C = max(max(counts), 128)
    CP = _round_up(C, 128)

    CPX = _round_up(CP, 256)  # x block grid is 256-wide

    if C not in _PROGRAM_CACHE:
        _PROGRAM_CACHE[C] = _build_program(C)
    nc = _PROGRAM_CACHE[C]

    wpacks = _pack_weights(w1, w2, nf8)

    in_maps = []
    for e in range(N_EXPERTS):
        n = counts[e]
        xt = np.zeros((D_MODEL, CPX), dtype=np.float32)
        if n:
            xt[:, :n] = x2d[tokens[e]].T
        xh, xl = _split8(xt, nf8)
        m = {
            "xh": _pack_x_blocks(xh),
            "xl": _pack_x_blocks(xl),
        }
        m.update(wpacks[e])
        in_maps.append(m)

    res = run_bass_kernel_spmd(nc, in_maps, core_ids=list(range(N_CORES)))

    out2d = np.zeros((T, D_MODEL), dtype=np.float32)
    for e in range(N_EXPERTS):
        n = counts[e]
        if n:
            # y comes back transposed [d_model, tokens] and unscaled; apply
            # combine weight and the deferred 1/WS here.
            yt = res.results[e]["y"].reshape(D_MODEL, -1)[:, :n]
            out2d[tokens[e]] += (yt.astype(np.float32)
                                 * (weights[e] / WS)[None, :]).T

    LAST_BUILD["nc"] = nc
    LAST_BUILD["C"] = C
    return out2d.reshape(B, S, D_MODEL)
